# revision 37
# baseline (speedup 1.0000x reference)
"""CRAFT OHEM loss on 8 trn2 NeuronCores — data-parallel over batch.

Math: with uniform-random inputs, n_neg_total (≈0.25·N) is always far below
NEG_RATIO·n_pos (≈2.25·N), so the reference's OHEM top-k selects *all*
negatives and every branch of the loss reduces to masked global sums:

    pos  = (region_target > .5) | (affinity_target > .5)   [= max(rt,at) > .5]
    S_pos_r = Σ pos·(rp-rt)²    S_tot_r = Σ (rp-rt)²       (same for affinity)
    n_pos   = Σ pos             n_neg_tot = N - n_pos

    region_loss   = S_pos_r/n_pos + (S_tot_r - S_pos_r)/n_neg
    affinity_loss = S_pos_a/n_pos + (S_tot_a - S_pos_a)/n_neg

Each core streams its 4-image shard once; the run is DMA-bound (26.2 MB/core
at the 360 GB/s bus = 72.8 us, plus ~1.9 us issue lead-in).  The host
interleaves the four maps chunk-major into ONE packed input tensor; steady
1600-col chunks use one DMA per tensor-quarter so quarters land early and
compute pipelines into the transfer window.

Only DVE and ACT carry compute (neuronxcc cannot encode elementwise work on
the Pool engine, and tensor_scalar's accumulator is broken on hardware).
Steady chunks split per-element work as: DVE max/sub/sub + the region
masked sum as an fp32 scalar_tensor_tensor (accumulate verified on HW);
the affinity masked sum runs as q = sign·sq in bf16 — the one DVE form the
cost model gives a 2x dtype speedup — with an ACT Copy-accumulate of q, and
the host recovers S_pos_a = (Σ s·sq_a + S_tot_a)/2.  ACT also emits
sign(mx-0.5) (n_pos via Σsign) and both squares (accumulators sum pre-cast
fp32 values even with bf16 outputs).  A shrinking DMA-split taper
(800/400/300 cols) plus a final 100-col chunk whose whole ladder runs on
DVE keeps the post-stream drain short, and the stats writeback is split so
only a 5-column DMA trails the last compute.  The host combines all
per-(core,chunk) partials in float64 and falls back to an exact numpy OHEM
in the (unreachable for this input distribution) case
n_neg_tot > NEG_RATIO·n_pos.
"""

import numpy as np

import concourse.bass as bass
import concourse.bacc as bacc
import concourse.mybir as mybir
from concourse.tile import TileContext
from concourse.bass_utils import run_bass_kernel_spmd

N_CORES = 8
B, H, W = 32, 640, 640
N_TOTAL = B * H * W                  # 13_107_200
PER_CORE = N_TOTAL // N_CORES        # 1_638_400
P = 128
F_TOT = PER_CORE // P                # 12_800
NEG_RATIO = 3.0

# (f, mode) per chunk.  mode: "quad"  = 4 per-tensor DMAs, 3-engine split
#                             "packed"= 1 DMA, 3-engine split
#                             "split" = 2 DMAs (targets|preds), 3-engine split
#                             "dve"   = 2 DMAs, whole ladder on DVE
PLAN = [(1600, "quad")] * 7 \
    + [(800, "splitx"), (400, "splitx"), (300, "splitx"), (100, "dve")]
BULK_AT = 8                          # stats writeback point (chunk index)
assert sum(f for f, _ in PLAN) == F_TOT
NSTAT = 5                            # pos_r, pos_a, tot_r, tot_a, n_pos/sign

_F32 = mybir.dt.float32
_BF16 = mybir.dt.bfloat16


def build_nc(plan=None, bulk_at=None, pio_bufs=3, mid_bufs=2, io_bufs=3, defer=False, inplace_sq=False, hold_last_k=0) -> bass.Bass:
    if plan is None:
        plan = PLAN
        bulk_at = BULK_AT
    chunks = [f for f, _ in plan]
    nchunk = len(plan)

    nc = bacc.Bacc(None)
    # packed chunk-major input: chunk i occupies cols [4*off_i, 4*off_i+4f)
    # as [rt | at | rp | ap], each f cols.
    pk = nc.dram_tensor("packed", [P, 4 * F_TOT], _BF16, kind="ExternalInput")
    # chunk-major stats: cols [5i, 5i+5) = chunk i's
    #   [Σpos·sq_r, Σpos·sq_a, Σsq_r, Σsq_a, n_pos]
    st_out = nc.dram_tensor(
        "stats", [P, NSTAT * nchunk], _F32, kind="ExternalOutput"
    )

    SQ = mybir.ActivationFunctionType.Square
    SIGN = mybir.ActivationFunctionType.Sign
    COPY = mybir.ActivationFunctionType.Copy
    IS_GT = mybir.AluOpType.is_gt
    BYPASS = mybir.AluOpType.bypass
    MULT = mybir.AluOpType.mult

    f_max = max(chunks)

    with TileContext(nc) as tc:
        with tc.tile_pool(name="io", bufs=io_bufs) as io, \
             tc.tile_pool(name="pio", bufs=pio_bufs) as pio, \
             tc.tile_pool(name="mid", bufs=mid_bufs) as mid, \
             tc.tile_pool(name="hold", bufs=1) as hold, \
             tc.tile_pool(name="fix", bufs=1) as fix:
            st = fix.tile([P, NSTAT * nchunk], _F32)
            scr_v = fix.tile([P, f_max], _F32)     # DVE stt garbage out
            scr_b = fix.tile([P, f_max], _BF16)    # ACT copy garbage out
            neg_half = fix.tile([P, 1], _F32)      # bias for sign(mx - 0.5)
            nc.vector.memset(neg_half[:], -0.5)

            off = 0
            prev_quad = False
            bulk_done = 0
            pending = []
            held = []
            n_quad = sum(1 for _, m in plan if m == "quad")
            for i, (f, mode) in enumerate(plan):
                held_chunk = mode == "quad" and i >= n_quad - hold_last_k
                base = 4 * off
                off += f
                c0 = NSTAT * i

                if mode in ("quad", "quadb", "quadv"):
                    quarters = []
                    for k, tag in enumerate(("rt", "at", "rp", "ap")):
                        q = io.tile([P, f], _BF16, tag=tag)
                        nc.sync.dma_start(
                            out=q[:], in_=pk[:, base + k * f : base + (k + 1) * f]
                        )
                        quarters.append(q[:])
                    rt_t, at_t, rp_t, ap_t = quarters
                else:
                    in_t = pio.tile([P, 4 * f], _BF16, tag="in")
                    dmae = nc.scalar if mode.endswith("_act") else nc.sync
                    if mode.startswith("packed"):
                        dmae.dma_start(
                            out=in_t[:], in_=pk[:, base : base + 4 * f]
                        )
                    else:  # split/dve/vx...: targets land before preds
                        dmae.dma_start(
                            out=in_t[:, : 2 * f], in_=pk[:, base : base + 2 * f]
                        )
                        dmae.dma_start(
                            out=in_t[:, 2 * f :],
                            in_=pk[:, base + 2 * f : base + 4 * f],
                        )
                    rt_t = in_t[:, 0 * f : 1 * f]
                    at_t = in_t[:, 1 * f : 2 * f]
                    rp_t = in_t[:, 2 * f : 3 * f]
                    ap_t = in_t[:, 3 * f : 4 * f]

                if mode != "quad":
                    # all deferred back-ops must land before any non-quad
                    # chunk allocates mid tiles, or the pool recycles their
                    # still-unread inputs
                    while pending:
                        pending.pop(0)()

                pool_of = (lambda tag: (hold, f"h{i}_" + tag)) if held_chunk \
                    else (lambda tag: (mid, tag))

                _p, _t = pool_of("mx")
                mx = _p.tile([P, f], _BF16, tag=_t)
                nc.vector.tensor_max(mx[:], rt_t, at_t)

                # s = sign(mx-0.5) on ACT: exact ±1/0 mask encoding (fp32
                # compare, bf16 storage), accum st4 = Σsign = 2·n_pos − n.
                # mx−0.5 is exact by Sterbenz for mx ∈ [0.25,1] and
                # sign-safe below that.
                _p, _t = pool_of("s")
                s_bf = _p.tile([P, f], _BF16, tag=_t)
                nc.scalar.activation(
                    s_bf[:], mx[:], SIGN, bias=neg_half[:],
                    accum_out=st[:, c0 + 4 : c0 + 5],
                )

                dr = mid.tile([P, f], _BF16, tag="dr")
                nc.vector.tensor_sub(dr[:], rp_t, rt_t)
                da = mid.tile([P, f], _BF16, tag="da")
                nc.vector.tensor_sub(da[:], ap_t, at_t)
                del _p, _t

                if mode.startswith("dve"):
                    # last chunk: everything after the preds DMA on DVE, no
                    # cross-engine hops (sign above is off the critical path)
                    sqr = mid.tile([P, f], _F32, tag="sqr")
                    nc.vector.scalar_tensor_tensor(
                        sqr[:], dr[:], 0.0, dr[:], op0=BYPASS, op1=MULT,
                        accum_out=st[:, c0 + 2 : c0 + 3],
                    )
                    sqa = mid.tile([P, f], _F32, tag="sqa")
                    nc.vector.scalar_tensor_tensor(
                        sqa[:], da[:], 0.0, da[:], op0=BYPASS, op1=MULT,
                        accum_out=st[:, c0 + 3 : c0 + 4],
                    )
                    nc.vector.scalar_tensor_tensor(
                        scr_v[:, :f], mx[:], 0.5, sqr[:],
                        op0=IS_GT, op1=MULT, accum_out=st[:, c0 : c0 + 1],
                    )
                    nc.vector.scalar_tensor_tensor(
                        scr_v[:, :f], mx[:], 0.5, sqa[:],
                        op0=IS_GT, op1=MULT, accum_out=st[:, c0 + 1 : c0 + 2],
                    )
                elif mode == "quadb":
                    # full-bf16 steady: both masked sums via q = s·sq (DVE 2x)
                    # + ACT Copy accums; host halves st0 and st1
                    sqr_bf = mid.tile([P, f], _BF16, tag="sqr")
                    nc.scalar.activation(
                        sqr_bf[:], dr[:], SQ, accum_out=st[:, c0 + 2 : c0 + 3]
                    )
                    sqa_bf = mid.tile([P, f], _BF16, tag="sqa")
                    nc.scalar.activation(
                        sqa_bf[:], da[:], SQ, accum_out=st[:, c0 + 3 : c0 + 4]
                    )
                    q_r = mid.tile([P, f], _BF16, tag="qr")
                    nc.vector.tensor_tensor(
                        out=q_r[:], in0=s_bf[:], in1=sqr_bf[:], op=MULT
                    )
                    nc.scalar.activation(
                        scr_b[:, :f], q_r[:], COPY,
                        accum_out=st[:, c0 : c0 + 1],
                    )
                    q_a = mid.tile([P, f], _BF16, tag="qa")
                    nc.vector.tensor_tensor(
                        out=q_a[:], in0=s_bf[:], in1=sqa_bf[:], op=MULT
                    )
                    nc.scalar.activation(
                        scr_b[:, :f], q_a[:], COPY,
                        accum_out=st[:, c0 + 1 : c0 + 2],
                    )
                elif mode in ("quad", "splitx"):
                    # steady state, balanced DVE/ACT:
                    #   region: fp32 STT masked sum on DVE (st0 direct)
                    #   affinity: q = s·sq_a in bf16 (DVE 2x mode), summed by
                    #   an ACT Copy accum (st1 = Σ s·sq_a; host halves it)
                    if held_chunk:
                        sqr = hold.tile([P, f], _BF16, tag=f"h{i}_sqr")
                    elif inplace_sq:
                        sqr = dr
                    else:
                        sqr = mid.tile([P, f], _BF16, tag="sqr")
                    nc.scalar.activation(
                        sqr[:], dr[:], SQ, accum_out=st[:, c0 + 2 : c0 + 3]
                    )
                    if held_chunk:
                        sqa_bf = hold.tile([P, f], _BF16, tag=f"h{i}_sqa")
                    else:
                        sqa_bf = mid.tile([P, f], _BF16, tag="sqa")
                    nc.scalar.activation(
                        sqa_bf[:], da[:], SQ, accum_out=st[:, c0 + 3 : c0 + 4]
                    )

                    def back(mx=mx, s_bf=s_bf, sqr=sqr, sqa_bf=sqa_bf,
                             c0=c0, f=f):
                        nc.vector.scalar_tensor_tensor(
                            scr_v[:, :f], mx[:], 0.5, sqr[:],
                            op0=IS_GT, op1=MULT, accum_out=st[:, c0 : c0 + 1],
                        )
                        if inplace_sq:
                            q_a = s_bf
                        else:
                            q_a = mid.tile([P, f], _BF16, tag="qa")
                        nc.vector.tensor_tensor(
                            out=q_a[:], in0=s_bf[:], in1=sqa_bf[:], op=MULT
                        )
                        nc.scalar.activation(
                            scr_b[:, :f], q_a[:], COPY,
                            accum_out=st[:, c0 + 1 : c0 + 2],
                        )

                    if held_chunk:
                        held.append(back)
                    elif defer:
                        pending.append(back)
                    else:
                        back()
                else:
                    # taper: DVE does both masked sums (f is small); ACT the
                    # squares
                    sqr = mid.tile([P, f], _BF16, tag="sqr")
                    nc.scalar.activation(
                        sqr[:], dr[:], SQ, accum_out=st[:, c0 + 2 : c0 + 3]
                    )
                    sqa = mid.tile([P, f], _BF16, tag="sqaf")
                    nc.scalar.activation(
                        sqa[:], da[:], SQ, accum_out=st[:, c0 + 3 : c0 + 4]
                    )
                    nc.vector.scalar_tensor_tensor(
                        scr_v[:, :f], mx[:], 0.5, sqr[:],
                        op0=IS_GT, op1=MULT, accum_out=st[:, c0 : c0 + 1],
                    )
                    nc.vector.scalar_tensor_tensor(
                        scr_v[:, :f], mx[:], 0.5, sqa[:],
                        op0=IS_GT, op1=MULT, accum_out=st[:, c0 + 1 : c0 + 2],
                    )

                while len(pending) > 1:
                    pending.pop(0)()
                if i == nchunk - 1:
                    while pending:
                        pending.pop(0)()

                # bulk stats writeback overlaps the tail chunks' DMA/compute;
                # only the last chunk's 5 columns remain for the final DMA.
                if i == (bulk_at if bulk_at is not None else nchunk - 2):
                    nc.sync.dma_start(
                        out=st_out[:, : c0 + NSTAT], in_=st[:, : c0 + NSTAT]
                    )
                    bulk_done = c0 + NSTAT
            for back in held:
                back()
            nc.sync.dma_start(out=st_out[:, bulk_done:], in_=st[:, bulk_done:])
    nc.compile()
    return nc


_NC_CACHE: dict = {}


def _get_nc() -> bass.Bass:
    if "nc" not in _NC_CACHE:
        _NC_CACHE["nc"] = build_nc()
    return _NC_CACHE["nc"]


def _shard(x: np.ndarray, c: int) -> np.ndarray:
    per_b = B // N_CORES
    return np.ascontiguousarray(x.reshape(B, H * W)[c * per_b : (c + 1) * per_b]).reshape(
        P, F_TOT
    )


def _pack(rt: np.ndarray, at: np.ndarray, rp: np.ndarray, ap: np.ndarray) -> np.ndarray:
    """Chunk-major interleave of the four [P, F_TOT] shards, cast to bf16.

    bf16 halves the HBM traffic (the kernel is DMA-bound in fp32) and keeps
    every accumulator in fp32 on-device; the loss error this introduces is
    ~1e-4 relative — far inside the 2e-2 gate."""
    bf16 = mybir.dt.np(mybir.dt.bfloat16)
    parts = []
    off = 0
    for f, _ in PLAN:
        sl = slice(off, off + f)
        off += f
        parts += [rt[:, sl], at[:, sl], rp[:, sl], ap[:, sl]]
    return np.ascontiguousarray(np.concatenate(parts, axis=1)).astype(bf16)


def _host_fallback_topk(region_pred, affinity_pred, region_target, affinity_target,
                        n_pos, n_neg):
    """Exact OHEM (reference semantics) on host — unreachable for uniform data."""
    rlm = (region_pred.astype(np.float64) - region_target.astype(np.float64)) ** 2
    alm = (affinity_pred.astype(np.float64) - affinity_target.astype(np.float64)) ** 2
    pos = (region_target > 0.5) | (affinity_target > 0.5)
    neg = ~pos
    comb = ((rlm + alm) * neg).reshape(-1)
    idx = np.argsort(-comb, kind="stable")[:n_neg]
    neg_r = rlm.reshape(-1)[idx].mean()
    neg_a = alm.reshape(-1)[idx].mean()
    pos_r = (rlm * pos).sum() / n_pos
    pos_a = (alm * pos).sum() / n_pos
    return pos_r + neg_r, pos_a + neg_a


def kernel(region_pred, affinity_pred, region_target, affinity_target):
    region_pred = np.asarray(region_pred, dtype=np.float32)
    affinity_pred = np.asarray(affinity_pred, dtype=np.float32)
    region_target = np.asarray(region_target, dtype=np.float32)
    affinity_target = np.asarray(affinity_target, dtype=np.float32)

    nc = _get_nc()
    in_maps = [
        {
            "packed": _pack(
                _shard(region_target, c),
                _shard(affinity_target, c),
                _shard(region_pred, c),
                _shard(affinity_pred, c),
            ),
        }
        for c in range(N_CORES)
    ]
    res = run_bass_kernel_spmd(nc, in_maps, list(range(N_CORES))).results

    nchunk = len(PLAN)
    S_pos_r = S_pos_a = S_tot_r = S_tot_a = sign_sum = 0.0
    for c in range(N_CORES):
        st = res[c]["stats"].astype(np.float64).reshape(P, nchunk, NSTAT)
        S_tot_r += st[:, :, 2].sum()
        S_tot_a += st[:, :, 3].sum()
        sign_sum += st[:, :, 4].sum()
        s0 = st[:, :, 0].sum(axis=0)
        s1 = st[:, :, 1].sum(axis=0)
        s2 = st[:, :, 2].sum(axis=0)
        s3 = st[:, :, 3].sum(axis=0)
        S_pos_r_half = {}
        for i, (f, mode) in enumerate(PLAN):
            if mode in ("quad", "quadb", "splitx"):
                S_pos_a += (s1[i] + s3[i]) / 2.0   # Σ s·sq_a -> masked sum
            else:
                S_pos_a += s1[i]                   # direct masked sum
            if mode == "quadb":
                S_pos_r += (s0[i] + s2[i]) / 2.0
            else:
                S_pos_r += s0[i]

    # Σ sign(mx-0.5) = n_pos − n_neg = 2·n_pos − N
    n_pos = int(round((sign_sum + N_TOTAL) / 2.0))
    n_neg_tot = N_TOTAL - n_pos

    if n_pos == 0:
        region_loss = S_tot_r / N_TOTAL
        affinity_loss = S_tot_a / N_TOTAL
    else:
        pos_r = S_pos_r / n_pos
        pos_a = S_pos_a / n_pos
        n_neg = min(n_neg_tot, int(n_pos * NEG_RATIO))
        if n_neg == 0:
            region_loss, affinity_loss = pos_r, pos_a
        elif n_neg == n_neg_tot:
            region_loss = pos_r + (S_tot_r - S_pos_r) / n_neg
            affinity_loss = pos_a + (S_tot_a - S_pos_a) / n_neg
        else:
            region_loss, affinity_loss = _host_fallback_topk(
                region_pred, affinity_pred, region_target, affinity_target,
                n_pos, n_neg,
            )

    total = np.float32(region_loss + affinity_loss)
    return (total, np.float32(region_loss), np.float32(affinity_loss))
